# revision 5
# baseline (speedup 1.0000x reference)
"""Elementwise scale kernel: out = x * w  (x: [16,4096,4096] f32, w: [4096] f32).

Data-parallel across 8 NeuronCores: shard x along batch dim (2 per core),
replicate w. Per core: 64 tiles of [128, 4096] f32; load on the SP HWDGE
ring, multiply by a partition-broadcast w tile on DVE, store on the ACT
HWDGE ring so loads and stores queue on independent FIFOs.
"""

import numpy as np

import concourse.bass as bass
import concourse.mybir as mybir
from concourse.bass_utils import run_bass_kernel_spmd
from concourse.tile import TileContext

BATCH, SEQ, ISIZE = 16, 4096, 4096
N_CORES = 8
B_LOC = BATCH // N_CORES          # 2 batch rows per core
ROWS = B_LOC * SEQ                # 8192
P = 128                           # SBUF partitions
N_TILES = ROWS // P               # 64
BUFS = 8

_NC = None


def _split_multi_waits(nc):
    """Walrus codegen embeds at most one sync-wait per instruction; Tile can
    attach several. Hoist extras onto standalone event-semaphore pseudo-ops
    executed by the same engine's sequencer immediately before."""
    idx = 0
    for f in nc.m.functions:
        for blk in f.blocks:
            new_list = []
            changed = False
            for ins in blk.instructions:
                si = getattr(ins, "sync_info", None)
                if si is not None and si.on_wait and len(si.on_wait) > 1:
                    changed = True
                    for w in si.on_wait[:-1]:
                        ev = mybir.InstEventSemaphore(
                            name=f"waitsplit-{idx}", ins=[], outs=[]
                        )
                        idx += 1
                        ev.engine = ins.engine
                        ev.sync_info = mybir.SyncInfo(on_wait=[w], on_update=[])
                        new_list.append(ev)
                    ins.sync_info = mybir.SyncInfo(
                        on_wait=[si.on_wait[-1]], on_update=si.on_update
                    )
                new_list.append(ins)
            if changed:
                try:
                    blk.instructions = new_list
                except AttributeError:
                    blk.instructions[:] = new_list


def _build():
    nc = bass.Bass()
    x_in = nc.declare_dram_parameter("x", [ROWS, ISIZE], mybir.dt.float32, isOutput=False)
    w_in = nc.declare_dram_parameter("w", [ISIZE], mybir.dt.float32, isOutput=False)
    out = nc.declare_dram_parameter("out", [ROWS, ISIZE], mybir.dt.float32, isOutput=True)

    with TileContext(nc) as tc:
        with (
            tc.tile_pool(name="wpool", bufs=1) as wpool,
            tc.tile_pool(name="xpool", bufs=BUFS) as xpool,
        ):
            w_tile = wpool.tile([P, ISIZE], mybir.dt.float32)
            nc.gpsimd.dma_start(out=w_tile[:], in_=w_in[None, :].to_broadcast((P, ISIZE)))
            # Make DVE observe the w-broadcast semaphore once, so the per-tile
            # muls carry a single sync wait (the TT ISA slot limit).
            scratch = wpool.tile([P, 1], mybir.dt.float32)
            nc.vector.tensor_copy(out=scratch[:], in_=w_tile[:, 0:1])
            for t in range(N_TILES):
                x_tile = xpool.tile([P, ISIZE], mybir.dt.float32)
                rows = x_in[t * P : (t + 1) * P, :]
                nc.sync.dma_start(out=x_tile[:], in_=rows)
                nc.vector.tensor_mul(out=x_tile[:], in0=x_tile[:], in1=w_tile[:])
                nc.scalar.dma_start(out=out[t * P : (t + 1) * P, :], in_=x_tile[:])
    _split_multi_waits(nc)
    return nc


def _get_nc():
    global _NC
    if _NC is None:
        _NC = _build()
    return _NC


def kernel(x: np.ndarray, w: np.ndarray, _results_out: list | None = None) -> np.ndarray:
    x = np.ascontiguousarray(x, dtype=np.float32)
    w = np.ascontiguousarray(w, dtype=np.float32)
    nc = _get_nc()
    in_maps = [
        {"x": x[c * B_LOC : (c + 1) * B_LOC].reshape(ROWS, ISIZE), "w": w}
        for c in range(N_CORES)
    ]
    res = run_bass_kernel_spmd(nc, in_maps, list(range(N_CORES)))
    if _results_out is not None:
        _results_out.append(res)
    out = np.empty((BATCH, SEQ, ISIZE), dtype=np.float32)
    for c in range(N_CORES):
        out[c * B_LOC : (c + 1) * B_LOC] = res.results[c]["out"].reshape(B_LOC, SEQ, ISIZE)
    return out


# revision 6
# speedup vs baseline: 1.5573x; 1.5573x over previous
"""Elementwise scale kernel: out = x * w  (x: [16,4096,4096] f32, w: [4096] f32).

Data-parallel across 8 NeuronCores: shard x along the batch dim (2 rows of
the leading dim per core), replicate w. Per core the 128 MiB shard moves as
16 tiles of [128 partitions x 16384 f32] (8 MiB per DMA, 64 KiB per
partition — the largest clean descriptor size). Loads issue on the SP HWDGE
ring, stores on the ACT HWDGE ring; DVE multiplies in place against a
partition-broadcast w tile. On this execution path each dma_start carries a
large fixed cost, so few/large DMAs win over many/small ones.
"""

import numpy as np

import concourse.bass as bass
import concourse.mybir as mybir
from concourse.bass_utils import run_bass_kernel_spmd
from concourse.tile import TileContext

BATCH, SEQ, ISIZE = 16, 4096, 4096
N_CORES = 8
B_LOC = BATCH // N_CORES          # 2 batch rows per core
ROWS = B_LOC * SEQ                # 8192
P = 128                           # SBUF partitions
FREE = 16384                      # f32 elements per partition per tile
TILE_ELEMS = P * FREE
N_TILES = (ROWS * ISIZE) // TILE_ELEMS  # 16
BUFS = 2

_NC = None


def _split_multi_waits(nc):
    """Walrus codegen embeds at most one sync-wait per instruction; Tile can
    attach several. Hoist extras onto standalone event-semaphore pseudo-ops
    executed by the same engine's sequencer immediately before."""
    idx = 0
    for f in nc.m.functions:
        for blk in f.blocks:
            new_list = []
            changed = False
            for ins in blk.instructions:
                si = getattr(ins, "sync_info", None)
                if si is not None and si.on_wait and len(si.on_wait) > 1:
                    changed = True
                    for w in si.on_wait[:-1]:
                        ev = mybir.InstEventSemaphore(
                            name=f"waitsplit-{idx}", ins=[], outs=[]
                        )
                        idx += 1
                        ev.engine = ins.engine
                        ev.sync_info = mybir.SyncInfo(on_wait=[w], on_update=[])
                        new_list.append(ev)
                    ins.sync_info = mybir.SyncInfo(
                        on_wait=[si.on_wait[-1]], on_update=si.on_update
                    )
                new_list.append(ins)
            if changed:
                try:
                    blk.instructions = new_list
                except AttributeError:
                    blk.instructions[:] = new_list


def _build():
    nc = bass.Bass()
    x_in = nc.declare_dram_parameter(
        "x", [ROWS * ISIZE], mybir.dt.float32, isOutput=False
    )
    w_in = nc.declare_dram_parameter("w", [ISIZE], mybir.dt.float32, isOutput=False)
    out = nc.declare_dram_parameter(
        "out", [ROWS * ISIZE], mybir.dt.float32, isOutput=True
    )

    with TileContext(nc) as tc:
        with (
            tc.tile_pool(name="wpool", bufs=1) as wpool,
            tc.tile_pool(name="xpool", bufs=BUFS) as xpool,
        ):
            w_tile = wpool.tile([P, ISIZE], mybir.dt.float32)
            nc.gpsimd.dma_start(
                out=w_tile[:], in_=w_in[None, :].to_broadcast((P, ISIZE))
            )
            # In-place touch: DVE observes the w-broadcast semaphore once, so
            # the per-tile muls carry a single sync wait (TT ISA wait limit).
            nc.vector.tensor_copy(out=w_tile[:, 0:1], in_=w_tile[:, 0:1])
            for t in range(N_TILES):
                x_tile = xpool.tile([P, FREE], mybir.dt.float32)
                src = x_in[t * TILE_ELEMS : (t + 1) * TILE_ELEMS].rearrange(
                    "(p f) -> p f", p=P
                )
                dst = out[t * TILE_ELEMS : (t + 1) * TILE_ELEMS].rearrange(
                    "(p f) -> p f", p=P
                )
                nc.sync.dma_start(out=x_tile[:], in_=src)
                for r in range(FREE // ISIZE):
                    nc.vector.tensor_mul(
                        out=x_tile[:, r * ISIZE : (r + 1) * ISIZE],
                        in0=x_tile[:, r * ISIZE : (r + 1) * ISIZE],
                        in1=w_tile[:],
                    )
                nc.scalar.dma_start(out=dst, in_=x_tile[:])
    _split_multi_waits(nc)
    return nc


def _get_nc():
    global _NC
    if _NC is None:
        _NC = _build()
    return _NC


def kernel(x: np.ndarray, w: np.ndarray, _results_out: list | None = None) -> np.ndarray:
    x = np.ascontiguousarray(x, dtype=np.float32)
    w = np.ascontiguousarray(w, dtype=np.float32)
    nc = _get_nc()
    in_maps = [
        {"x": x[c * B_LOC : (c + 1) * B_LOC].reshape(ROWS * ISIZE), "w": w}
        for c in range(N_CORES)
    ]
    res = run_bass_kernel_spmd(nc, in_maps, list(range(N_CORES)))
    if _results_out is not None:
        _results_out.append(res)
    out = np.empty((BATCH, SEQ, ISIZE), dtype=np.float32)
    for c in range(N_CORES):
        out[c * B_LOC : (c + 1) * B_LOC] = res.results[c]["out"].reshape(
            B_LOC, SEQ, ISIZE
        )
    return out


# revision 7
# speedup vs baseline: 2.1099x; 1.3548x over previous
"""Elementwise scale kernel: out = x * w  (x: [16,4096,4096] f32, w: [4096] f32).

Data-parallel across 8 NeuronCores: shard x along the batch dim (2 rows of
the leading dim per core), replicate w. On this execution path each
dma_start carries a large fixed per-instruction cost, so the 128 MiB shard
moves in as few DMAs as possible: one [128, 16384] tile plus 10 merged
groups, each a single 2-chunk DMA over a 3-D tile [128, 2, 12296] (two
48 KiB-per-partition chunks — the largest descriptor size that doesn't
split — with an 8-element gap so the access pattern stays non-collapsible).
That is 11 loads + 11 stores = 22 DMAs instead of 32. Loads issue on the SP
HWDGE ring, stores on the ACT ring; DVE multiplies in place against a
partition-broadcast w tile. dynamic_dma_scratch_size=16000 (vs default
16384) frees just enough SBUF to double-buffer the 96 KiB merged groups.
"""

import numpy as np

import concourse.bass as bass
import concourse.mybir as mybir
from concourse.bass_utils import run_bass_kernel_spmd
from concourse.tile import TileContext

BATCH, SEQ, ISIZE = 16, 4096, 4096
N_CORES = 8
B_LOC = BATCH // N_CORES          # 2 batch rows per core
ROWS = B_LOC * SEQ                # 8192
P = 128                           # SBUF partitions
FREE = 16384                      # single-tile width (f32 elems/partition)
CH = 12288                        # merged-group chunk width (48 KiB)
GAP = 8                           # keeps the 2-chunk AP non-collapsible
N_GROUPS = 10
BUFS = 2

_NC = None


def _split_multi_waits(nc):
    """Walrus codegen embeds at most one sync-wait per instruction; Tile can
    attach several. Hoist extras onto standalone event-semaphore pseudo-ops
    executed by the same engine's sequencer immediately before."""
    idx = 0
    for f in nc.m.functions:
        for blk in f.blocks:
            new_list = []
            changed = False
            for ins in blk.instructions:
                si = getattr(ins, "sync_info", None)
                if si is not None and si.on_wait and len(si.on_wait) > 1:
                    changed = True
                    for w in si.on_wait[:-1]:
                        ev = mybir.InstEventSemaphore(
                            name=f"waitsplit-{idx}", ins=[], outs=[]
                        )
                        idx += 1
                        ev.engine = ins.engine
                        ev.sync_info = mybir.SyncInfo(on_wait=[w], on_update=[])
                        new_list.append(ev)
                    ins.sync_info = mybir.SyncInfo(
                        on_wait=[si.on_wait[-1]], on_update=si.on_update
                    )
                new_list.append(ins)
            if changed:
                try:
                    blk.instructions = new_list
                except AttributeError:
                    blk.instructions[:] = new_list


def _build():
    nc = bass.Bass(dynamic_dma_scratch_size=16000)
    x_in = nc.declare_dram_parameter(
        "x", [ROWS * ISIZE], mybir.dt.float32, isOutput=False
    )
    w_in = nc.declare_dram_parameter("w", [ISIZE], mybir.dt.float32, isOutput=False)
    out = nc.declare_dram_parameter(
        "out", [ROWS * ISIZE], mybir.dt.float32, isOutput=True
    )

    with TileContext(nc) as tc:
        with (
            tc.tile_pool(name="wpool", bufs=1) as wpool,
            tc.tile_pool(name="xpool", bufs=BUFS) as xpool,
        ):
            w_tile = wpool.tile([P, ISIZE], mybir.dt.float32)
            nc.gpsimd.dma_start(
                out=w_tile[:], in_=w_in[None, :].to_broadcast((P, ISIZE))
            )
            # In-place touch: DVE observes the w-broadcast semaphore once, so
            # the per-tile muls carry a single sync wait (TT ISA wait limit).
            nc.vector.tensor_copy(out=w_tile[:, 0:1], in_=w_tile[:, 0:1])

            off = 0
            x_t = xpool.tile([P, FREE], mybir.dt.float32, tag="g")
            tb = P * FREE
            nc.sync.dma_start(
                out=x_t[:], in_=x_in[off : off + tb].rearrange("(p f) -> p f", p=P)
            )
            for k in range(FREE // ISIZE):
                sl = x_t[:, k * ISIZE : (k + 1) * ISIZE]
                nc.vector.tensor_mul(out=sl, in0=sl, in1=w_tile[:])
            nc.scalar.dma_start(
                out=out[off : off + tb].rearrange("(p f) -> p f", p=P), in_=x_t[:]
            )
            off += tb

            gb = 2 * P * CH
            for _ in range(N_GROUPS):
                t3 = xpool.tile([P, 2, CH + GAP], mybir.dt.float32, tag="g")
                s3 = x_in[off : off + gb].rearrange("(c p f) -> p c f", c=2, p=P, f=CH)
                d3 = out[off : off + gb].rearrange("(c p f) -> p c f", c=2, p=P, f=CH)
                nc.sync.dma_start(out=t3[:, :, 0:CH], in_=s3)
                for c in range(2):
                    for k in range(CH // ISIZE):
                        sl = t3[:, c, k * ISIZE : (k + 1) * ISIZE]
                        nc.vector.tensor_mul(out=sl, in0=sl, in1=w_tile[:])
                nc.scalar.dma_start(out=d3, in_=t3[:, :, 0:CH])
                off += gb
            assert off == ROWS * ISIZE
    _split_multi_waits(nc)
    return nc


def _get_nc():
    global _NC
    if _NC is None:
        _NC = _build()
    return _NC


def kernel(x: np.ndarray, w: np.ndarray, _results_out: list | None = None) -> np.ndarray:
    x = np.ascontiguousarray(x, dtype=np.float32)
    w = np.ascontiguousarray(w, dtype=np.float32)
    nc = _get_nc()
    in_maps = [
        {"x": x[c * B_LOC : (c + 1) * B_LOC].reshape(ROWS * ISIZE), "w": w}
        for c in range(N_CORES)
    ]
    res = run_bass_kernel_spmd(nc, in_maps, list(range(N_CORES)))
    if _results_out is not None:
        _results_out.append(res)
    out = np.empty((BATCH, SEQ, ISIZE), dtype=np.float32)
    for c in range(N_CORES):
        out[c * B_LOC : (c + 1) * B_LOC] = res.results[c]["out"].reshape(
            B_LOC, SEQ, ISIZE
        )
    return out
